# revision 2
# baseline (speedup 1.0000x reference)
"""DETM decoder kernel for 8 Trainium2 NeuronCores.

Computes word_dist[b, v] = sum_k theta[b,k] * softmax_v(alpha[b,k,:] @ rho[v,:]^T)
for B=128, K=50, L=300, V=50000.

Strategy:
  - Shard V across the 8 cores (6250 columns each). alpha/theta replicated.
  - Rows are (k, b) k-major: row-block k holds all 128 b's for topic k, so the
    final k-sum is a per-partition weighted accumulation.
  - Single fp32r matmul pass computes logit tiles; a fused ScalarE activation
    does exp(logit - c[row]) with a per-row safe shift c (host-computed upper
    bound on the row max) and accumulates the softmax denominator Z for free.
  - exp tiles (P) are stashed to local DRAM in bf16 (80 MB/core).
  - Z partials are AllReduced across cores (25.6 KB), then w = theta / Z.
  - Second pass reloads P and computes out[b,v] = sum_k w[k,b] * P[k][b,v] as
    50 PSUM-accumulated diag(w_k) matmuls per output tile.
"""
import sys
import functools

if "/opt/trn_rl_repo" not in sys.path:
    sys.path.insert(0, "/opt/trn_rl_repo")

import numpy as np
import ml_dtypes

import concourse.bass as bass  # noqa: F401  (engine types resolve through bacc)
from concourse import bacc, mybir, bass_utils
import concourse.tile as tile

B, K, L, V = 128, 50, 300, 50000
NCORES = 8
VC = V // NCORES          # 6250 columns per core
ROWS = K * B              # 6400 (row = k*128 + b)

F32 = mybir.dt.float32
F32R = mybir.dt.float32r
BF16 = mybir.dt.bfloat16
Exp = mybir.ActivationFunctionType.Exp

LCHUNKS = [(0, 128), (128, 128), (256, 44)]          # L = 300
VTILES = [(i * 512, 512) for i in range(12)] + [(6144, 106)]
VGROUPS = [(0, 2048), (2048, 2048), (4096, 2048), (6144, 106)]


def _body(nc, tc, dram, io):
    """One full computation of the output (repeatable for timing)."""
    from contextlib import ExitStack

    with ExitStack() as ctx:
        const = ctx.enter_context(tc.tile_pool(name="const", bufs=1))

        # ---- resident inputs ----
        at_sb = []
        rt_sb = []
        for lc, (l0, ln) in enumerate(LCHUNKS):
            t = const.tile([ln, ROWS], F32R, tag=f"at{lc}")
            nc.sync.dma_start(t[:], io["alphaT"][l0:l0 + ln, :])
            at_sb.append(t)
            r = const.tile([ln, VC], F32R, tag=f"rt{lc}")
            nc.sync.dma_start(r[:], io["rhoT"][l0:l0 + ln, :])
            rt_sb.append(r)
        cneg_sb = const.tile([B, K], F32)
        nc.sync.dma_start(cneg_sb[:], io["cneg"])
        th_sb = const.tile([B, K], F32)
        nc.sync.dma_start(th_sb[:], io["thetaT"])
        id_sb = const.tile([B, B], BF16)
        nc.sync.dma_start(id_sb[:], io["ident"])

        zp = const.tile([B, K * 13], F32)       # per-(k, vtile) partial Z
        zall = const.tile([B, K], F32)
        rz = const.tile([B, K], F32)
        w_sb = const.tile([B, K], F32)
        wk_sb = const.tile([B, K, B], BF16)     # 50 diag(w_k) matrices

        pstash = dram.tile([K, B, VC], BF16)
        zin_d = dram.tile([B, K], F32)
        zout_d = dram.tile([B, K], F32)

        # ================= phase 1: logits -> exp -> stash + Z =============
        with tc.tile_pool(name="ps1", bufs=4, space="PSUM") as ps1, \
             tc.tile_pool(name="pstage", bufs=3) as pstage:
            for k in range(K):
                kcol = slice(k * B, (k + 1) * B)
                pst = None
                for vi, (v0, vn) in enumerate(VTILES):
                    ps = ps1.tile([B, vn], F32, tag="ps")
                    for lc in range(3):
                        nc.tensor.matmul(
                            ps[:], at_sb[lc][:, kcol], rt_sb[lc][:, v0:v0 + vn],
                            start=(lc == 0), stop=(lc == 2))
                    gi = vi // 4
                    g0, gn = VGROUPS[gi]
                    if vi % 4 == 0:
                        pst = pstage.tile([B, gn], BF16, tag="pst")
                    nc.scalar.activation(
                        pst[:, v0 - g0:v0 - g0 + vn], ps[:], Exp,
                        bias=cneg_sb[:, k:k + 1], scale=1.0,
                        accum_out=zp[:, k * 13 + vi:k * 13 + vi + 1])
                    if vi % 4 == 3 or vi == 12:
                        nc.sync.dma_start(pstash[k][:, g0:g0 + gn], pst[:])

        # ================= Z allreduce + weights ===========================
        zv = zp[:].rearrange("p (k t) -> p k t", k=K)
        nc.vector.tensor_reduce(zall[:], zv, axis=mybir.AxisListType.X,
                                op=mybir.AluOpType.add)
        nc.sync.dma_start(zin_d[:], zall[:])
        nc.gpsimd.collective_compute(
            "AllReduce", mybir.AluOpType.add,
            replica_groups=[list(range(NCORES))],
            ins=[zin_d.opt()], outs=[zout_d.opt()])
        nc.sync.dma_start(zall[:], zout_d[:])
        nc.vector.reciprocal(rz[:], zall[:])
        nc.vector.tensor_mul(w_sb[:], th_sb[:], rz[:])
        for k in range(K):
            nc.vector.tensor_scalar_mul(wk_sb[:, k, :], id_sb[:],
                                        w_sb[:, k:k + 1])

        # ================= phase 2: out = sum_k diag(w_k) @ P_k ============
        with tc.tile_pool(name="ps2", bufs=8, space="PSUM") as ps2, \
             tc.tile_pool(name="ld", bufs=3) as ldp, \
             tc.tile_pool(name="ost", bufs=2) as ostp:
            for g0, gn in VGROUPS:
                nvt = (gn + 511) // 512
                pss = [ps2.tile([B, min(512, gn - j * 512)], F32, tag="acc",
                                name=f"acc{j}")
                       for j in range(nvt)]
                for k in range(K):
                    ld = ldp.tile([B, gn], BF16, tag="ld")
                    nc.sync.dma_start(ld[:], pstash[k][:, g0:g0 + gn])
                    for j in range(nvt):
                        jn = min(512, gn - j * 512)
                        nc.tensor.matmul(
                            pss[j][:], wk_sb[:, k, :],
                            ld[:, j * 512:j * 512 + jn],
                            start=(k == 0), stop=(k == K - 1))
                ot = ostp.tile([B, gn], F32, tag="ot")
                for j in range(nvt):
                    jn = min(512, gn - j * 512)
                    nc.scalar.copy(ot[:, j * 512:j * 512 + jn], pss[j][:])
                nc.sync.dma_start(io["out"][:, g0:g0 + gn], ot[:])


@functools.lru_cache(maxsize=2)
def _build(reps=1):
    nc = bacc.Bacc("TRN2", target_bir_lowering=False, debug=False,
                   num_devices=NCORES)
    io = {
        "alphaT": nc.dram_tensor("alphaT", [L, ROWS], F32R,
                                 kind="ExternalInput").ap(),
        "rhoT": nc.dram_tensor("rhoT", [L, VC], F32R,
                               kind="ExternalInput").ap(),
        "cneg": nc.dram_tensor("cneg", [B, K], F32,
                               kind="ExternalInput").ap(),
        "thetaT": nc.dram_tensor("thetaT", [B, K], F32,
                                 kind="ExternalInput").ap(),
        "ident": nc.dram_tensor("ident", [B, B], BF16,
                                kind="ExternalInput").ap(),
        "out": nc.dram_tensor("out", [B, VC], F32,
                              kind="ExternalOutput").ap(),
    }
    with tile.TileContext(nc) as tc:
        with tc.tile_pool(name="dram", bufs=1, space="DRAM") as dram:
            for _ in range(reps):
                _body(nc, tc, dram, io)
    nc.compile()
    return nc, io


def _host_prep(theta, alpha, word_embeddings):
    theta = np.ascontiguousarray(theta, dtype=np.float32)
    alpha = np.ascontiguousarray(alpha, dtype=np.float32)
    we = np.ascontiguousarray(word_embeddings, dtype=np.float32)

    alphaT = np.ascontiguousarray(
        alpha.transpose(2, 1, 0).reshape(L, ROWS))       # col = k*128 + b
    rhoT = np.ascontiguousarray(we.T)                    # (L, V)
    # per-(b,k) safe shift: upper bound on max_v logits, see module docstring
    cneg = -(4.65 * np.linalg.norm(alpha, axis=2) + 10.0).astype(np.float32)
    ident = np.eye(B, dtype=ml_dtypes.bfloat16)

    in_maps = []
    for c in range(NCORES):
        in_maps.append({
            "alphaT": alphaT,
            "rhoT": np.ascontiguousarray(rhoT[:, c * VC:(c + 1) * VC]),
            "cneg": cneg,
            "thetaT": theta,
            "ident": ident,
        })
    return in_maps


def run_on_cores(theta, alpha, word_embeddings, reps=1):
    nc, io = _build(reps)
    in_maps = _host_prep(theta, alpha, word_embeddings)
    res = bass_utils.run_bass_kernel_spmd(nc, in_maps,
                                          core_ids=list(range(NCORES)))
    out = np.concatenate([res.results[c]["out"] for c in range(NCORES)],
                         axis=1)
    return out.astype(np.float32)


def kernel(theta, alpha, word_embeddings):
    return run_on_cores(theta, alpha, word_embeddings, reps=1)


# revision 4
# speedup vs baseline: 1.9455x; 1.9455x over previous
"""DETM decoder kernel for 8 Trainium2 NeuronCores.

Computes word_dist[b, v] = sum_k theta[b,k] * softmax_v(alpha[b,k,:] @ rho[v,:]^T)
for B=128, K=50, L=300, V=50000.

The execution backend charges a large, payload-insensitive cost per
instruction, so the design minimizes instruction count and keeps same-engine
instructions in long blocks:
  - Shard V across the 8 cores (6656 padded columns each; V padded 50000 ->
    53248 with zero embedding rows whose exp(0 - c) underflows to 0).
  - Rows are (k, b) k-major so row-block k holds all 128 b's for topic k.
  - Per topic k: 39 back-to-back fp32r matmuls fill all 8 PSUM banks in two
    half-rounds; 4 big fused exp instructions (bias = -c[row] safe shift,
    accum_out = partial Z) move exp(logits) to SBUF in bf16; 1 DMA stashes
    P_k [128, 6656] to DRAM scratch.
  - One 25.6 KB AllReduce combines Z across cores; w = theta / Z.
  - Phase 2: 50 DMA reloads + 50 big DVE scalar_tensor_tensor instructions
    accumulate out[b,v] = sum_k w[k,b] * P_k[b,v]; 1 DMA writes the shard.
~2.3k instructions total.
"""
import sys
import functools

if "/opt/trn_rl_repo" not in sys.path:
    sys.path.insert(0, "/opt/trn_rl_repo")

import numpy as np
import ml_dtypes

from concourse import bacc, mybir, bass_utils
import concourse.tile as tile

B, K, L, V = 128, 50, 300, 50000
NCORES = 8
VC = 6656                 # padded columns per core (13 x 512)
VPAD = VC * NCORES        # 53248
ROWS = K * B              # 6400 (row = k*128 + b)

F32 = mybir.dt.float32
F32R = mybir.dt.float32r
BF16 = mybir.dt.bfloat16
Exp = mybir.ActivationFunctionType.Exp

LCHUNKS = [(0, 128), (128, 128), (256, 44)]          # L = 300
NVT = VC // 512                                      # 13 matmul tiles per row
# PSUM is filled in half-rounds of 4 banks -> one exp each
EXPGROUPS = [(0, 2048), (2048, 2048), (4096, 2048), (6144, 512)]


def _body(nc, tc, dram, io):
    """One full computation of the output (repeatable for timing)."""
    from contextlib import ExitStack

    with ExitStack() as ctx:
        res = ctx.enter_context(tc.tile_pool(name="res", bufs=1))

        cneg_sb = res.tile([B, K], F32)
        nc.sync.dma_start(cneg_sb[:], io["cneg"])
        th_sb = res.tile([B, K], F32)
        nc.sync.dma_start(th_sb[:], io["thetaT"])
        zp = res.tile([B, K * 4], F32)        # per-(k, expgroup) partial Z
        zall = res.tile([B, K], F32)
        rz = res.tile([B, K], F32)
        w_sb = res.tile([B, K], F32)

        pstash = dram.tile([K, B, VC], BF16)
        zin_d = dram.tile([B, K], F32)
        zout_d = dram.tile([B, K], F32)

        # ================= phase 1: logits -> exp -> stash + Z =============
        with tc.tile_pool(name="mats", bufs=1) as mats, \
             tc.tile_pool(name="ps1", bufs=2, space="PSUM") as ps1, \
             tc.tile_pool(name="pstage", bufs=2) as pstage:
            at_sb = []
            rt_sb = []
            for lc, (l0, ln) in enumerate(LCHUNKS):
                t = mats.tile([ln, ROWS], F32R, tag=f"at{lc}", name=f"at{lc}")
                nc.sync.dma_start(t[:], io["alphaT"][l0:l0 + ln, :])
                at_sb.append(t)
                r = mats.tile([ln, VC], F32R, tag=f"rt{lc}", name=f"rt{lc}")
                nc.sync.dma_start(r[:], io["rhoT"][l0:l0 + ln, :])
                rt_sb.append(r)

            for k in range(K):
                kcol = slice(k * B, (k + 1) * B)
                pst = pstage.tile([B, VC], BF16, tag="pst", name="pst")
                # rounds of up to 4 PSUM banks, one big exp per round
                for gi, (g0, gn) in enumerate(EXPGROUPS):
                    psh = ps1.tile([B, gn], F32, tag="psh", name="psh")
                    for j in range(gn // 512):
                        v0 = g0 + j * 512
                        for lc in range(3):
                            nc.tensor.matmul(
                                psh[:, j * 512:(j + 1) * 512],
                                at_sb[lc][:, kcol],
                                rt_sb[lc][:, v0:v0 + 512],
                                start=(lc == 0), stop=(lc == 2))
                    nc.scalar.activation(
                        pst[:, g0:g0 + gn], psh[:], Exp,
                        bias=cneg_sb[:, k:k + 1], scale=1.0,
                        accum_out=zp[:, k * 4 + gi:k * 4 + gi + 1])
                nc.sync.dma_start(pstash[k], pst[:])

        # ================= Z allreduce + weights ===========================
        zv = zp[:].rearrange("p (k t) -> p k t", k=K)
        nc.vector.tensor_reduce(zall[:], zv, axis=mybir.AxisListType.X,
                                op=mybir.AluOpType.add)
        nc.sync.dma_start(zin_d[:], zall[:])
        nc.gpsimd.collective_compute(
            "AllReduce", mybir.AluOpType.add,
            replica_groups=[list(range(NCORES))],
            ins=[zin_d.opt()], outs=[zout_d.opt()])
        nc.sync.dma_start(zall[:], zout_d[:])
        nc.vector.reciprocal(rz[:], zall[:])
        nc.vector.tensor_mul(w_sb[:], th_sb[:], rz[:])

        # ========= phase 2: out[b,v] = sum_k w[k,b] * P_k[b,v] =============
        with tc.tile_pool(name="ld", bufs=2) as ldp, \
             tc.tile_pool(name="accp", bufs=1) as accp:
            acc = accp.tile([B, VC], F32)
            for k in range(K):
                ld = ldp.tile([B, VC], BF16, tag="ld", name="ld")
                nc.sync.dma_start(ld[:], pstash[k])
                if k == 0:
                    nc.vector.tensor_scalar_mul(acc[:], ld[:], w_sb[:, 0:1])
                else:
                    nc.vector.scalar_tensor_tensor(
                        acc[:], ld[:], w_sb[:, k:k + 1], acc[:],
                        op0=mybir.AluOpType.mult, op1=mybir.AluOpType.add)
            nc.sync.dma_start(io["out"], acc[:])


@functools.lru_cache(maxsize=2)
def _build(reps=1):
    nc = bacc.Bacc("TRN2", target_bir_lowering=False, debug=False,
                   num_devices=NCORES)
    io = {
        "alphaT": nc.dram_tensor("alphaT", [L, ROWS], F32R,
                                 kind="ExternalInput").ap(),
        "rhoT": nc.dram_tensor("rhoT", [L, VC], F32R,
                               kind="ExternalInput").ap(),
        "cneg": nc.dram_tensor("cneg", [B, K], F32,
                               kind="ExternalInput").ap(),
        "thetaT": nc.dram_tensor("thetaT", [B, K], F32,
                                 kind="ExternalInput").ap(),
        "out": nc.dram_tensor("out", [B, VC], F32,
                              kind="ExternalOutput").ap(),
    }
    with tile.TileContext(nc) as tc:
        with tc.tile_pool(name="dram", bufs=1, space="DRAM") as dram:
            for _ in range(reps):
                _body(nc, tc, dram, io)
    nc.compile()
    return nc, io


def _host_prep(theta, alpha, word_embeddings):
    theta = np.ascontiguousarray(theta, dtype=np.float32)
    alpha = np.ascontiguousarray(alpha, dtype=np.float32)
    we = np.ascontiguousarray(word_embeddings, dtype=np.float32)

    alphaT = np.ascontiguousarray(
        alpha.transpose(2, 1, 0).reshape(L, ROWS))       # col = k*128 + b
    rhoT = np.zeros((L, VPAD), np.float32)
    rhoT[:, :V] = we.T
    # per-(b,k) safe shift: statistical upper bound on max_v logits
    cneg = -(4.65 * np.linalg.norm(alpha, axis=2) + 10.0).astype(np.float32)

    in_maps = []
    for c in range(NCORES):
        in_maps.append({
            "alphaT": alphaT,
            "rhoT": np.ascontiguousarray(rhoT[:, c * VC:(c + 1) * VC]),
            "cneg": cneg,
            "thetaT": theta,
        })
    return in_maps


def run_on_cores(theta, alpha, word_embeddings, reps=1):
    nc, io = _build(reps)
    in_maps = _host_prep(theta, alpha, word_embeddings)
    res = bass_utils.run_bass_kernel_spmd(nc, in_maps,
                                          core_ids=list(range(NCORES)))
    out = np.concatenate([res.results[c]["out"] for c in range(NCORES)],
                         axis=1)
    return out[:, :V].astype(np.float32)


def kernel(theta, alpha, word_embeddings):
    return run_on_cores(theta, alpha, word_embeddings, reps=1)
